# revision 17
# baseline (speedup 1.0000x reference)
"""Two-layer GraphConv (DGL norm='both') on 8 Trainium2 NeuronCores.

Strategy (dst-sharded graph parallel, window-major sweeps):
  - Nodes split into 8 contiguous shards of 12500; core c owns dst-shard c and
    the ~200k edges whose dst lands in it.
  - Per layer: each core computes hW = (h * norm_src) @ W for its own 12500
    nodes (f32 table: 512B gather descriptors are 2x faster per byte than
    256B); four fragment AllGathers assemble the table, issued as their
    producing tiles complete. A dummy collective fires first so the one-time
    ncfw rendezvous barrier overlaps the dense phase.
  - Aggregation runs in WINDOW-MAJOR sweeps: sweep w gathers and matmuls only
    window-w chunks for all tiles, accumulating partials in a bf16 SBUF
    accumulator. Sweep w starts as soon as fragment w's AllGather lands, so
    the serial collective stream pipelines with gather work instead of
    stalling the whole phase on the last fragment (the tile-major variant
    lost ~450us at layer entry and ~180us at the layer boundary to this).
  - Gathers: one dma_gather per (4-tile group, window) piece with a STATIC
    count (<=1024 indices -- the SWDGE ring limit; larger calls hang).
    Padding indices point at row 0 and are killed by all-zero one-hot
    columns; static counts need no per-call gpsimd reg_load (the baseline
    spent ~0.5ms there).
  - Segment-sum over dst on the TensorEngine: per 128-edge chunk a one-hot
    matrix O[e, dst_local] (is_equal of dst-local ids against an iota row) is
    matmul'd against the gathered rows, accumulating in PSUM per (tile, w).
  - Layer-2 AllGathers are issued inside layer-1's last sweep right after the
    group past each fragment's final producing tile.

One SPMD program runs on all cores; per-core graph structure lives in the
input data. Chunk capacities per (tile, window) are the max over the 8 cores.
"""

import os
import numpy as np
import ml_dtypes

N_NODES = 100000
N_EDGES = 1600000
D = 128
NC = 8
P = 128
SHARD = N_NODES // NC            # 12500
TILES = (SHARD + P - 1) // P     # 98 dst tiles/core (last tile 84 valid rows)
SHARD_PAD = TILES * P            # 12544
NW = 4
FR = SHARD // NW                 # 3125 local rows per fragment

T_GROUP = 4                      # dst tiles per gather call group
CALL_MAX_CH = int(os.environ.get("CCAS_MAXCH", "8"))
NQUEUES = 4

BF16 = ml_dtypes.bfloat16

_cache = {}


def _plan(src, dst):
    """Host-side graph partitioning -> structural plan + per-core data."""
    deg_out = np.bincount(src, minlength=N_NODES)
    deg_in = np.bincount(dst, minlength=N_NODES)
    norm_src = 1.0 / np.sqrt(np.maximum(deg_out, 1.0))
    norm_dst = 1.0 / np.sqrt(np.maximum(deg_in, 1.0))

    shard_of = dst // SHARD
    src_r = src // SHARD
    src_l = src % SHARD
    win_of = src_l // FR
    frag_row = src_r * FR + src_l % FR

    counts = np.zeros((NC, TILES, NW), np.int64)
    per_core = []
    for c in range(NC):
        m = shard_of == c
        es, ed, ew = frag_row[m], dst[m], win_of[m]
        dloc = ed - c * SHARD
        tl = dloc // P
        order = np.lexsort((es, ew, tl))
        es, ew, tl, dloc = es[order], ew[order], tl[order], dloc[order]
        np.add.at(counts[c], (tl, ew), 1)
        per_core.append((es, ew, tl, dloc))

    cap = counts.max(axis=0)                      # [TILES, NW]
    cap_ch = (-(-cap // P)).astype(np.int64)      # chunks per (tile, window)
    for t in range(TILES):
        if cap_ch[t].sum() == 0:
            cap_ch[t, 0] = 1

    groups = [list(range(g, min(g + T_GROUP, TILES)))
              for g in range(0, TILES, T_GROUP)]

    # global chunk layout: group-major, window-major, tile-minor, so each
    # (group, window) section is one contiguous chunk range
    chunk_pos = np.zeros((TILES, NW), np.int64)
    calls_gw = [[[] for _ in range(NW)] for _ in groups]
    sec_max = 0
    pos = 0
    for g, gts in enumerate(groups):
        for w in range(NW):
            sec0 = pos
            for t in gts:
                chunk_pos[t, w] = pos
                pos += int(cap_ch[t, w])
            sec_n = pos - sec0
            sec_max = max(sec_max, sec_n)
            off = 0
            while off < sec_n:
                piece = min(sec_n - off, CALL_MAX_CH)
                calls_gw[g][w].append((off, (sec0 + off) * 8, piece))
                off += piece
    total_chunks = pos
    idx_cols = total_chunks * 8

    # per-core arrays
    idx_all = np.zeros((NC, 16, idx_cols), np.int16)
    dstl_all = np.full((NC, P, total_chunks), -1.0, BF16)
    for c in range(NC):
        es, ew, tl, dloc = per_core[c]
        cnt = counts[c]
        pos_e = 0
        for t in range(TILES):
            for w in range(NW):
                n = int(cnt[t, w])
                lo, hi = pos_e, pos_e + n
                pos_e += n
                nch = int(cap_ch[t, w])
                if nch == 0:
                    continue
                cp = int(chunk_pos[t, w])
                seg = np.zeros(nch * P, np.int16)
                seg[:n] = es[lo:hi].astype(np.int16)
                idx_all[c, :, cp * 8:(cp + nch) * 8] = \
                    seg.reshape(nch * 8, 16).T
                dl = np.full(nch * P, -1.0, BF16)
                if n:
                    dl[:n] = (dloc[lo:hi] % P).astype(BF16)
                dstl_all[c, :, cp:cp + nch] = dl.reshape(nch, P).T

    def tilemajor(v, c):
        out = np.ones((SHARD_PAD,), np.float32)
        out[:SHARD] = v[c * SHARD:(c + 1) * SHARD]
        return np.ascontiguousarray(out.reshape(TILES, P).T)

    ns_tm = np.stack([tilemajor(norm_src, c) for c in range(NC)])
    nd_tm = np.stack([tilemajor(norm_dst, c) for c in range(NC)])

    plan = dict(groups=groups, calls_gw=calls_gw,
                cap_ch=cap_ch.tolist(), chunk_pos=chunk_pos.tolist(),
                sec_max=sec_max, total_chunks=total_chunks, idx_cols=idx_cols)
    data = dict(idx_all=idx_all, dstl_all=dstl_all,
                ns_tm=ns_tm, nd_tm=nd_tm)
    return plan, data


def _build(plan, with_bias):
    import concourse.bass as bass
    import concourse.mybir as mybir
    import concourse.tile as tile
    from concourse import bacc
    from concourse.masks import make_identity

    f32 = mybir.dt.float32
    gdt = f32

    groups = plan["groups"]
    calls_gw = plan["calls_gw"]
    cap_ch = plan["cap_ch"]
    chunk_pos = plan["chunk_pos"]
    sec_max = plan["sec_max"]
    idx_cols = plan["idx_cols"]
    total_chunks = plan["total_chunks"]
    bucket_max = max(max(r) for r in cap_ch)
    first_w = [min(w for w in range(NW) if cap_ch[t][w]) for t in range(TILES)]
    last_w = [max(w for w in range(NW) if cap_ch[t][w]) for t in range(TILES)]

    # last tile writing rows of fragment k (frag rows [k*FR,(k+1)*FR))
    frag_done_tile = [((k + 1) * FR - 1) // P for k in range(NW)]

    nc = bacc.Bacc("TRN2", target_bir_lowering=False, debug=False,
                   num_devices=NC, num_swdge_queues=NQUEUES)

    x_in = nc.dram_tensor("x_in", [SHARD_PAD, D], f32, kind="ExternalInput")
    w1_in = nc.dram_tensor("w1_in", [D, D], gdt, kind="ExternalInput")
    w2_in = nc.dram_tensor("w2_in", [D, D], gdt, kind="ExternalInput")
    idx_in = nc.dram_tensor("idx_in", [P, idx_cols], mybir.dt.int16, kind="ExternalInput")
    dstl_in = nc.dram_tensor("dstl_in", [P, total_chunks], mybir.dt.bfloat16, kind="ExternalInput")
    ns_in = nc.dram_tensor("ns_in", [P, TILES], f32, kind="ExternalInput")
    nd_in = nc.dram_tensor("nd_in", [P, TILES], f32, kind="ExternalInput")
    nds_in = nc.dram_tensor("nds_in", [P, TILES], f32, kind="ExternalInput")
    if with_bias:
        b1_in = nc.dram_tensor("b1_in", [P, D], f32, kind="ExternalInput")
        b2_in = nc.dram_tensor("b2_in", [P, D], f32, kind="ExternalInput")
    y_out = nc.dram_tensor("y_out", [SHARD, D], f32, kind="ExternalOutput")

    ag1_in = nc.dram_tensor("ag1_in", [SHARD, D], gdt, kind="Internal")
    ag2_in = nc.dram_tensor("ag2_in", [SHARD, D], gdt, kind="Internal")
    hw1_frag = [nc.dram_tensor(f"hw1_frag{k}", [NC * FR, D], gdt, kind="Internal",
                               addr_space="Shared") for k in range(NW)]
    hw2_frag = [nc.dram_tensor(f"hw2_frag{k}", [NC * FR, D], gdt, kind="Internal",
                               addr_space="Shared") for k in range(NW)]
    warm_in = nc.dram_tensor("warm_in", [1, D], gdt, kind="Internal")
    warm_out = nc.dram_tensor("warm_out", [NC, D], gdt, kind="Internal",
                              addr_space="Shared")

    RELU = mybir.ActivationFunctionType.Relu
    COPY = mybir.ActivationFunctionType.Copy

    def all_gather(src_t, dst_t, k):
        nc.gpsimd.collective_compute(
            "AllGather", mybir.AluOpType.bypass,
            replica_groups=[list(range(NC))],
            ins=[src_t[k * FR:(k + 1) * FR, :]], outs=[dst_t[k][:]])

    with tile.TileContext(nc) as tc:
        with (
            tc.tile_pool(name="const", bufs=1) as const,
            tc.tile_pool(name="xio", bufs=3) as xio,
            tc.tile_pool(name="gbuf", bufs=3) as gbuf,
            tc.tile_pool(name="obuf", bufs=4) as obuf,
            tc.tile_pool(name="ep", bufs=3) as ep,
            tc.tile_pool(name="ps_agg", bufs=3, space="PSUM") as ps_agg,
            tc.tile_pool(name="ps_tr", bufs=2, space="PSUM") as ps_tr,
            tc.tile_pool(name="ps_mm", bufs=2, space="PSUM") as ps_mm,
        ):
            # warm up the collective stream during P0
            nc.gpsimd.collective_compute(
                "AllGather", mybir.AluOpType.bypass,
                replica_groups=[list(range(NC))],
                ins=[warm_in[:]], outs=[warm_out[:]])

            # ---- constants ----
            idx_t = const.tile([P, idx_cols], mybir.dt.int16)
            nc.sync.dma_start(out=idx_t[:], in_=idx_in[:])
            dstl_t = const.tile([P, total_chunks], mybir.dt.bfloat16)
            nc.sync.dma_start(out=dstl_t[:], in_=dstl_in[:])
            ns_t = const.tile([P, TILES], f32)
            nc.sync.dma_start(out=ns_t[:], in_=ns_in[:])
            nd_t = const.tile([P, TILES], f32)
            nc.sync.dma_start(out=nd_t[:], in_=nd_in[:])
            nds_t = const.tile([P, TILES], f32)
            nc.sync.dma_start(out=nds_t[:], in_=nds_in[:])
            w1_t = const.tile([D, D], gdt)
            nc.sync.dma_start(out=w1_t[:], in_=w1_in[:])
            w2_t = const.tile([D, D], gdt)
            nc.sync.dma_start(out=w2_t[:], in_=w2_in[:])
            if with_bias:
                b1_t = const.tile([P, D], f32)
                nc.sync.dma_start(out=b1_t[:], in_=b1_in[:])
                b2_t = const.tile([P, D], f32)
                nc.sync.dma_start(out=b2_t[:], in_=b2_in[:])
            ident = const.tile([P, P], gdt)
            make_identity(nc, ident[:])
            iota_i = const.tile([P, P], mybir.dt.int32)
            nc.gpsimd.iota(iota_i[:], pattern=[[1, P]], base=0, channel_multiplier=0)
            iota_b = const.tile([P, P], mybir.dt.bfloat16)
            nc.vector.tensor_copy(out=iota_b[:], in_=iota_i[:])
            # bf16 partial-sum accumulator across window sweeps
            aggsb = const.tile([P, TILES, D], mybir.dt.bfloat16)

            # static gather-count registers, one per distinct size
            sizes = {pc * P for g in calls_gw for w in g for (_o, _c, pc) in w}
            cnt_reg = {s: nc.gpsimd.to_reg(s) for s in sorted(sizes)}

            def dense_mm(lhsT, w_t, ag_dst, t):
                mm = ps_mm.tile([P, D], f32, space="PSUM", tag="mm")
                nc.tensor.matmul(mm[:], lhsT=lhsT, rhs=w_t[:], start=True, stop=True)
                hw_sb = xio.tile([P, D], gdt, tag="hw_sb")
                nc.scalar.activation(hw_sb[:], mm[:], COPY)
                rows = min(SHARD - t * P, P)
                nc.sync.dma_start(out=ag_dst[t * P:t * P + rows, :], in_=hw_sb[:rows, :])

            # ---- P0: x -> scale -> transpose -> @W1 -> ag1_in ----
            for t in range(TILES):
                xt = xio.tile([P, D], f32, tag="x_f32")
                nc.sync.dma_start(out=xt[:], in_=x_in[t * P:(t + 1) * P, :])
                xs = xio.tile([P, D], gdt, tag="x_g")
                nc.vector.tensor_tensor(
                    out=xs[:], in0=xt[:], in1=ns_t[:, t:t + 1].to_broadcast([P, D]),
                    op=mybir.AluOpType.mult)
                tp = ps_tr.tile([P, P], gdt, space="PSUM", tag="tr")
                nc.tensor.transpose(tp[:], xs[:], ident[:])
                xT = xio.tile([P, P], gdt, tag="xT")
                nc.scalar.activation(xT[:], tp[:], COPY)
                dense_mm(xT[:], w1_t, ag1_in, t)
                for k in range(NW):
                    if frag_done_tile[k] == t:
                        all_gather(ag1_in, hw1_frag, k)

            qn = [0]

            def epilogue(zin, t, layer):
                if layer == 1:
                    t2 = ep.tile([P, D], gdt, tag="t2")
                    if with_bias:
                        z = ep.tile([P, D], f32, tag="z")
                        nc.vector.tensor_tensor(
                            out=z[:], in0=zin,
                            in1=nd_t[:, t:t + 1].to_broadcast([P, D]),
                            op=mybir.AluOpType.mult)
                        nc.vector.tensor_add(out=z[:], in0=z[:], in1=b1_t[:])
                        nc.scalar.activation(t2[:], z[:], RELU,
                                             scale=ns_t[:, t:t + 1])
                    else:
                        nc.scalar.activation(t2[:], zin, RELU,
                                             scale=nds_t[:, t:t + 1])
                    tp = ps_tr.tile([P, P], gdt, space="PSUM", tag="tr")
                    nc.tensor.transpose(tp[:], t2[:], ident[:])
                    t2T = ep.tile([P, P], gdt, tag="t2T")
                    nc.scalar.activation(t2T[:], tp[:], COPY)
                    dense_mm(t2T[:], w2_t, ag2_in, t)
                else:
                    y = ep.tile([P, D], f32, tag="y")
                    if with_bias:
                        z = ep.tile([P, D], f32, tag="z")
                        nc.vector.tensor_tensor(
                            out=z[:], in0=zin,
                            in1=nd_t[:, t:t + 1].to_broadcast([P, D]),
                            op=mybir.AluOpType.mult)
                        nc.vector.tensor_add(out=z[:], in0=z[:], in1=b2_t[:])
                        nc.scalar.activation(y[:], z[:], RELU)
                    else:
                        nc.scalar.activation(y[:], zin, RELU,
                                             scale=nd_t[:, t:t + 1])
                    rows = min(SHARD - t * P, P)
                    nc.sync.dma_start(out=y_out[t * P:t * P + rows, :],
                                      in_=y[:rows, :])

            def agg_phase(frags, layer):
                # AG2 issue points: in the last sweep, right after the group
                # past the one covering the fragment's final producing tile
                ag_after_group = {}
                if layer == 1:
                    for k in range(NW):
                        gi = min(frag_done_tile[k] // T_GROUP + 1, len(groups) - 1)
                        ag_after_group.setdefault(gi, []).append(k)
                for w in range(NW):
                    for g, gts in enumerate(groups):
                        if calls_gw[g][w]:
                            sec0 = chunk_pos[gts[0]][w]
                            G = gbuf.tile([P, sec_max, D], gdt, tag="G")
                            for (off, col0, piece) in calls_gw[g][w]:
                                nc.gpsimd.dma_gather(
                                    G[:, off:off + piece, :],
                                    frags[w][:],
                                    idx_t[:, col0:col0 + piece * 8],
                                    piece * P, cnt_reg[piece * P], D,
                                    queue_num=qn[0] % NQUEUES)
                                qn[0] += 1
                        for t in gts:
                            nch = cap_ch[t][w]
                            if nch == 0:
                                continue
                            cp = chunk_pos[t][w]
                            O = obuf.tile([P, bucket_max, P], gdt, tag="O")
                            nc.vector.tensor_tensor(
                                out=O[:, :nch, :],
                                in0=dstl_t[:, cp:cp + nch].unsqueeze(2).to_broadcast([P, nch, P]),
                                in1=iota_b[:].unsqueeze(1).to_broadcast([P, nch, P]),
                                op=mybir.AluOpType.is_equal)
                            agg = ps_agg.tile([P, D], f32, space="PSUM", tag="agg")
                            for jj in range(nch):
                                nc.tensor.matmul(
                                    agg[:], lhsT=O[:, jj, :],
                                    rhs=G[:, cp - sec0 + jj, :],
                                    start=(jj == 0), stop=(jj == nch - 1))
                            if first_w[t] == last_w[t] == w:
                                epilogue(agg[:], t, layer)
                            elif w == first_w[t]:
                                nc.scalar.activation(aggsb[:, t, :], agg[:], COPY)
                            elif w < last_w[t]:
                                nc.vector.tensor_add(
                                    out=aggsb[:, t, :], in0=agg[:], in1=aggsb[:, t, :])
                            else:
                                z = ep.tile([P, D], f32, tag="zf")
                                nc.vector.tensor_add(
                                    out=z[:], in0=agg[:], in1=aggsb[:, t, :])
                                epilogue(z[:], t, layer)
                        if layer == 1 and w == NW - 1:
                            for k in ag_after_group.get(g, ()):
                                all_gather(ag2_in, hw2_frag, k)

            phases = int(os.environ.get("CCAS_PHASES", "5"))
            if phases >= 3:
                agg_phase(hw1_frag, layer=1)
            if phases >= 5:
                agg_phase(hw2_frag, layer=2)

    nc.compile()
    return nc


def kernel(x, W1, b1, W2, b2, src, dst):
    from concourse.bass_utils import run_bass_kernel_spmd

    src = np.asarray(src).astype(np.int64)
    dst = np.asarray(dst).astype(np.int64)
    x = np.asarray(x, dtype=np.float32)
    W1 = np.asarray(W1, dtype=np.float32)
    W2 = np.asarray(W2, dtype=np.float32)
    b1 = np.asarray(b1, dtype=np.float32)
    b2 = np.asarray(b2, dtype=np.float32)

    plan, data = _plan(src, dst)
    with_bias = bool(np.any(b1) or np.any(b2))

    key = (with_bias, os.environ.get("CCAS_PHASES", "5"),
           repr(plan["calls_gw"]), repr(plan["cap_ch"]))
    key = hash(key)
    if key not in _cache:
        _cache[key] = _build(plan, with_bias)
    nc = _cache[key]

    in_maps = []
    for c in range(NC):
        xp = np.zeros((SHARD_PAD, D), np.float32)
        xp[:SHARD] = x[c * SHARD:(c + 1) * SHARD]
        m = dict(
            x_in=xp,
            w1_in=W1,
            w2_in=W2,
            idx_in=np.tile(data["idx_all"][c], (8, 1)),
            dstl_in=data["dstl_all"][c],
            ns_in=data["ns_tm"][c],
            nd_in=data["nd_tm"][c],
            nds_in=data["nd_tm"][c] * data["ns_tm"][c],
        )
        if with_bias:
            m["b1_in"] = np.broadcast_to(b1, (P, D)).astype(np.float32).copy()
            m["b2_in"] = np.broadcast_to(b2, (P, D)).astype(np.float32).copy()
        in_maps.append(m)

    prof_dir = os.environ.get("CCAS_PROFILE_DIR")
    if prof_dir:
        import sys, types
        if "antenv.axon_hooks" not in sys.modules:
            import antenv
            mod = types.ModuleType("antenv.axon_hooks")
            mod._hook = None
            mod.set_axon_ntff_profile_hook = lambda h: setattr(mod, "_hook", h)
            mod.get_axon_ntff_profile_hook = lambda: mod._hook
            sys.modules["antenv.axon_hooks"] = mod
            antenv.axon_hooks = mod
            from trn_agent_boot.trn_boot import _ntff_profile_via_ctypes
            mod.set_axon_ntff_profile_hook(
                _ntff_profile_via_ctypes("/opt/axon/libaxon_pjrt.so"))
        from antenv.axon_hooks import get_axon_ntff_profile_hook
        res = run_bass_kernel_spmd(nc, in_maps, core_ids=list(range(NC)))
        hook = get_axon_ntff_profile_hook()
        with hook(prof_dir, list(range(NC))):
            res = run_bass_kernel_spmd(nc, in_maps, core_ids=list(range(NC)))
    else:
        res = run_bass_kernel_spmd(nc, in_maps, core_ids=list(range(NC)))

    return np.concatenate([res.results[c]["y_out"] for c in range(NC)], axis=0)


# revision 18
# speedup vs baseline: 1.9181x; 1.9181x over previous
"""Two-layer GraphConv (DGL norm='both') on 8 Trainium2 NeuronCores.

Strategy (dst-sharded graph parallel):
  - Nodes split into 8 contiguous shards of 12500; core c owns dst-shard c and
    the ~200k edges whose dst lands in it.
  - Per layer: each core computes hW = (h * norm_src) @ W for its own 12500
    nodes, then an AllGather assembles the full 100k x 128 table in every
    core's DRAM.
  - Per-edge rows hW[src] are fetched with the GPSIMD bulk-gather (dma_gather;
    int16 indices relative to one of four <=32768-row windows of the table;
    max 1024 indices per call, calls striped over 4 SWDGE queues).
  - Segment-sum over dst runs on the TensorEngine: per 128-edge chunk a
    one-hot matrix O[e, dst_local] (VectorEngine is_equal of the dst-local ids
    against an iota row) is matmul'd against the gathered rows, accumulating
    in PSUM over the chunks of one dst tile.
  - Epilogue per dst tile on the ScalarEngine: relu(agg * scale), the scale
    folding norm_dst (and the next layer's norm_src) into one activation.

One SPMD program runs on all cores; per-core graph structure lives in the
input data. Chunk capacities per (dst-tile, window) are the max over the 8
cores; a core fills its real indices and pads with -1 (the gather ucode trims
trailing negatives, and stale slots are killed by one-hot rows of zeros).
"""

import os
import numpy as np
import ml_dtypes

N_NODES = 100000
N_EDGES = 1600000
D = 128
NC = 8
P = 128
SHARD = N_NODES // NC            # 12500
TILES = (SHARD + P - 1) // P     # 98 dst tiles/core (last tile 84 valid rows)
SHARD_PAD = TILES * P            # 12544
# the table is fragmented: fragment k holds local rows [k*FR,(k+1)*FR) of every
# shard, rank-major ([NC*FR, D] per fragment). Fragments double as the int16
# gather windows (NC*FR = 25000 <= 32768) and let each AllGather overlap the
# gather-bound aggregation phase (range-based deps).
NW = 4
FR = SHARD // NW                 # 3125 local rows per fragment

CALL_MAX_CH = 8                  # dma_gather ucode limit: 1024 idxs/call
NQUEUES = 4

BF16 = ml_dtypes.bfloat16

_cache = {}


def _plan(src, dst):
    """Host-side graph partitioning -> structural plan + per-core data."""
    deg_out = np.bincount(src, minlength=N_NODES)
    deg_in = np.bincount(dst, minlength=N_NODES)
    norm_src = 1.0 / np.sqrt(np.maximum(deg_out, 1.0))
    norm_dst = 1.0 / np.sqrt(np.maximum(deg_in, 1.0))

    shard_of = dst // SHARD
    src_r = src // SHARD
    src_l = src % SHARD
    win_of = src_l // FR
    frag_row = src_r * FR + src_l % FR

    counts = np.zeros((NC, TILES, NW), np.int64)
    per_core = []
    for c in range(NC):
        m = shard_of == c
        es, ed, ew = frag_row[m], dst[m], win_of[m]
        dloc = ed - c * SHARD
        tl = dloc // P
        order = np.lexsort((es, ew, tl))
        es, ew, tl, dloc = es[order], ew[order], tl[order], dloc[order]
        np.add.at(counts[c], (tl, ew), 1)
        per_core.append((es, ew, tl, dloc))

    cap = counts.max(axis=0)                      # [TILES, NW]
    cap_ch = (-(-cap // P)).astype(np.int64)      # chunks per (tile, window)

    # per-tile chunk layout: windows concatenated; every tile >=1 chunk
    ktile = cap_ch.sum(axis=1)
    for t in range(TILES):
        if ktile[t] == 0:
            cap_ch[t, 0] = 1
            ktile[t] = 1
    tile_ch0 = np.zeros(TILES + 1, np.int64)      # chunk offset of tile t
    np.cumsum(ktile, out=tile_ch0[1:])
    total_chunks = int(tile_ch0[-1])

    # gather calls: one per (tile, window [, piece of <=8 chunks])
    # (tile, window, chunk_off_in_tile, n_chunks, idx_col_off)
    calls = []
    idx_cols = 0
    for t in range(TILES):
        ch_in_tile = 0
        for w in range(NW):
            nch = int(cap_ch[t, w])
            while nch > 0:
                piece = min(nch, CALL_MAX_CH)
                calls.append((t, w, ch_in_tile, piece, idx_cols))
                idx_cols += piece * P // 16
                ch_in_tile += piece
                nch -= piece

    # per-core arrays
    n_calls = len(calls)
    idx_all = np.full((NC, 16, idx_cols), -1, np.int16)
    cnt_all = np.zeros((NC, 1, n_calls), np.int32)
    dstl_all = np.full((NC, P, total_chunks), -1.0, BF16)
    for c in range(NC):
        es, ew, tl, dloc = per_core[c]
        # bucket boundaries in (tile, window)-sorted edge order
        cnt = counts[c]
        pos = 0
        bnd = {}
        for t in range(TILES):
            for w in range(NW):
                n = int(cnt[t, w])
                bnd[(t, w)] = (pos, pos + n)
                pos += n
        # dstl: per (t, w) bucket occupies chunks at tile_ch0[t] + sum(cap_ch[t,:w])
        for t in range(TILES):
            choff = int(tile_ch0[t])
            for w in range(NW):
                nch = int(cap_ch[t, w])
                if nch == 0:
                    continue
                lo, hi = bnd[(t, w)]
                n = hi - lo
                dl = np.full(nch * P, -1.0, BF16)
                if n:
                    dl[:n] = (dloc[lo:hi] % P).astype(BF16)
                dstl_all[c, :, choff:choff + nch] = dl.reshape(nch, P).T
                choff += nch
        # idx: per call, real idxs then -1 fill
        for kcall, (t, w, ch0, piece, col0) in enumerate(calls):
            lo, hi = bnd[(t, w)]
            # chunk range of this piece within the (t, w) bucket
            wch0 = ch0 - int(cap_ch[t, :w].sum())  # piece offset inside bucket
            s0 = lo + wch0 * P
            s1 = min(hi, lo + (wch0 + piece) * P)
            n = max(0, s1 - s0)
            seg = np.full(piece * P, -1, np.int16)
            if n:
                seg[:n] = es[s0:s1].astype(np.int16)
            cnt_all[c, 0, kcall] = n
            idx_all[c, :, col0:col0 + piece * P // 16] = \
                seg.reshape(piece * P // 16, 16).T

    ktile_list = [int(k) for k in ktile]

    def tilemajor(v, c):
        out = np.ones((SHARD_PAD,), np.float32)
        out[:SHARD] = v[c * SHARD:(c + 1) * SHARD]
        return np.ascontiguousarray(out.reshape(TILES, P).T)

    ns_tm = np.stack([tilemajor(norm_src, c) for c in range(NC)])
    nd_tm = np.stack([tilemajor(norm_dst, c) for c in range(NC)])

    plan = dict(calls=calls, ktile=ktile_list,
                tile_ch0=[int(v) for v in tile_ch0],
                total_chunks=total_chunks, idx_cols=idx_cols)
    data = dict(idx_all=idx_all, cnt_all=cnt_all, dstl_all=dstl_all,
                ns_tm=ns_tm, nd_tm=nd_tm)
    return plan, data


def _build(plan, with_bias, use_bf16):
    import concourse.bass as bass
    import concourse.mybir as mybir
    import concourse.tile as tile
    from concourse import bacc
    from concourse.masks import make_identity

    f32 = mybir.dt.float32
    gdt = mybir.dt.bfloat16 if use_bf16 else f32

    calls = plan["calls"]
    ktile = plan["ktile"]
    tile_ch0 = plan["tile_ch0"]
    idx_cols = plan["idx_cols"]
    total_chunks = plan["total_chunks"]
    k_max = max(ktile)

    nc = bacc.Bacc("TRN2", target_bir_lowering=False, debug=False,
                   num_devices=NC, num_swdge_queues=NQUEUES)

    x_in = nc.dram_tensor("x_in", [SHARD_PAD, D], f32, kind="ExternalInput")
    w1_in = nc.dram_tensor("w1_in", [D, D], gdt, kind="ExternalInput")
    w2_in = nc.dram_tensor("w2_in", [D, D], gdt, kind="ExternalInput")
    idx_in = nc.dram_tensor("idx_in", [P, idx_cols], mybir.dt.int16, kind="ExternalInput")
    cnt_in = nc.dram_tensor("cnt_in", [1, len(calls)], mybir.dt.int32, kind="ExternalInput")
    dstl_in = nc.dram_tensor("dstl_in", [P, total_chunks], mybir.dt.bfloat16, kind="ExternalInput")
    ns_in = nc.dram_tensor("ns_in", [P, TILES], f32, kind="ExternalInput")
    nd_in = nc.dram_tensor("nd_in", [P, TILES], f32, kind="ExternalInput")
    nds_in = nc.dram_tensor("nds_in", [P, TILES], f32, kind="ExternalInput")
    if with_bias:
        b1_in = nc.dram_tensor("b1_in", [P, D], f32, kind="ExternalInput")
        b2_in = nc.dram_tensor("b2_in", [P, D], f32, kind="ExternalInput")
    y_out = nc.dram_tensor("y_out", [SHARD, D], f32, kind="ExternalOutput")

    ag1_in = nc.dram_tensor("ag1_in", [SHARD, D], gdt, kind="Internal")
    ag2_in = nc.dram_tensor("ag2_in", [SHARD, D], gdt, kind="Internal")
    hw1_frag = [nc.dram_tensor(f"hw1_frag{k}", [NC * FR, D], gdt, kind="Internal",
                               addr_space="Shared") for k in range(NW)]
    hw2_frag = [nc.dram_tensor(f"hw2_frag{k}", [NC * FR, D], gdt, kind="Internal",
                               addr_space="Shared") for k in range(NW)]

    RELU = mybir.ActivationFunctionType.Relu
    COPY = mybir.ActivationFunctionType.Copy

    with tile.TileContext(nc) as tc:
        with (
            tc.tile_pool(name="const", bufs=1) as const,
            tc.tile_pool(name="xio", bufs=3) as xio,
            tc.tile_pool(name="gbuf", bufs=6) as gbuf,
            tc.tile_pool(name="obuf", bufs=4) as obuf,
            tc.tile_pool(name="ep", bufs=3) as ep,
            tc.tile_pool(name="ps_agg", bufs=3, space="PSUM") as ps_agg,
            tc.tile_pool(name="ps_tr", bufs=2, space="PSUM") as ps_tr,
            tc.tile_pool(name="ps_mm", bufs=2, space="PSUM") as ps_mm,
        ):
            # ---- constants ----
            idx_t = const.tile([P, idx_cols], mybir.dt.int16)
            nc.sync.dma_start(out=idx_t[:], in_=idx_in[:])
            cnt_t = const.tile([1, len(calls)], mybir.dt.int32)
            nc.sync.dma_start(out=cnt_t[:], in_=cnt_in[:])
            dstl_t = const.tile([P, total_chunks], mybir.dt.bfloat16)
            nc.sync.dma_start(out=dstl_t[:], in_=dstl_in[:])
            ns_t = const.tile([P, TILES], f32)
            nc.sync.dma_start(out=ns_t[:], in_=ns_in[:])
            nd_t = const.tile([P, TILES], f32)
            nc.sync.dma_start(out=nd_t[:], in_=nd_in[:])
            nds_t = const.tile([P, TILES], f32)
            nc.sync.dma_start(out=nds_t[:], in_=nds_in[:])
            w1_t = const.tile([D, D], gdt)
            nc.sync.dma_start(out=w1_t[:], in_=w1_in[:])
            w2_t = const.tile([D, D], gdt)
            nc.sync.dma_start(out=w2_t[:], in_=w2_in[:])
            if with_bias:
                b1_t = const.tile([P, D], f32)
                nc.sync.dma_start(out=b1_t[:], in_=b1_in[:])
                b2_t = const.tile([P, D], f32)
                nc.sync.dma_start(out=b2_t[:], in_=b2_in[:])
            ident = const.tile([P, P], gdt)
            make_identity(nc, ident[:])
            iota_i = const.tile([P, P], mybir.dt.int32)
            nc.gpsimd.iota(iota_i[:], pattern=[[1, P]], base=0, channel_multiplier=0)
            iota_b = const.tile([P, P], mybir.dt.bfloat16)
            nc.vector.tensor_copy(out=iota_b[:], in_=iota_i[:])

            def dense_mm(lhsT, w_t, ag_dst, t):
                mm = ps_mm.tile([P, D], f32, space="PSUM", tag="mm")
                nc.tensor.matmul(mm[:], lhsT=lhsT, rhs=w_t[:], start=True, stop=True)
                hw_sb = xio.tile([P, D], gdt, tag="hw_sb")
                nc.scalar.activation(hw_sb[:], mm[:], COPY)
                rows = min(SHARD - t * P, P)
                nc.sync.dma_start(out=ag_dst[t * P:t * P + rows, :], in_=hw_sb[:rows, :])

            # ---- P0: x -> scale -> transpose -> @W1 -> ag1_in ----
            for t in range(TILES):
                xt = xio.tile([P, D], f32, tag="x_f32")
                nc.sync.dma_start(out=xt[:], in_=x_in[t * P:(t + 1) * P, :])
                xs = xio.tile([P, D], gdt, tag="x_g")
                nc.vector.tensor_tensor(
                    out=xs[:], in0=xt[:], in1=ns_t[:, t:t + 1].to_broadcast([P, D]),
                    op=mybir.AluOpType.mult)
                tp = ps_tr.tile([P, P], gdt, space="PSUM", tag="tr")
                nc.tensor.transpose(tp[:], xs[:], ident[:])
                xT = xio.tile([P, P], gdt, tag="xT")
                nc.scalar.activation(xT[:], tp[:], COPY)
                dense_mm(xT[:], w1_t, ag1_in, t)

            # ---- P1: per-fragment AllGathers (overlap with agg via deps) ----
            for k in range(NW):
                nc.gpsimd.collective_compute(
                    "AllGather", mybir.AluOpType.bypass,
                    replica_groups=[list(range(NC))],
                    ins=[ag1_in[k * FR:(k + 1) * FR, :]], outs=[hw1_frag[k][:]])

            qn = [0]

            def agg_phase(frags, layer):
                ci = 0
                for t in range(TILES):
                    kt = ktile[t]
                    oc0 = tile_ch0[t]
                    G = gbuf.tile([P, k_max, D], gdt, tag="G")
                    while ci < len(calls) and calls[ci][0] == t:
                        (_t, w, ch0, piece, col0) = calls[ci]
                        creg = nc.gpsimd.alloc_register()
                        nc.gpsimd.reg_load(creg, cnt_t[0:1, ci:ci + 1])
                        nc.gpsimd.dma_gather(
                            G[:, ch0:ch0 + piece, :],
                            frags[w][:],
                            idx_t[:, col0:col0 + piece * P // 16],
                            piece * P, creg, D,
                            queue_num=qn[0] % NQUEUES)
                        qn[0] += 1
                        ci += 1
                    O = obuf.tile([P, k_max, P], gdt, tag="O")
                    nc.vector.tensor_tensor(
                        out=O[:, :kt, :],
                        in0=dstl_t[:, oc0:oc0 + kt].unsqueeze(2).to_broadcast([P, kt, P]),
                        in1=iota_b[:].unsqueeze(1).to_broadcast([P, kt, P]),
                        op=mybir.AluOpType.is_equal)
                    agg = ps_agg.tile([P, D], f32, space="PSUM", tag="agg")
                    for j in range(kt):
                        nc.tensor.matmul(
                            agg[:], lhsT=O[:, j, :], rhs=G[:, j, :],
                            start=(j == 0), stop=(j == kt - 1))
                    if layer == 1:
                        t2 = ep.tile([P, D], gdt, tag="t2")
                        if with_bias:
                            z = ep.tile([P, D], f32, tag="z")
                            nc.vector.tensor_tensor(
                                out=z[:], in0=agg[:],
                                in1=nd_t[:, t:t + 1].to_broadcast([P, D]),
                                op=mybir.AluOpType.mult)
                            nc.vector.tensor_add(out=z[:], in0=z[:], in1=b1_t[:])
                            nc.scalar.activation(t2[:], z[:], RELU,
                                                 scale=ns_t[:, t:t + 1])
                        else:
                            nc.scalar.activation(t2[:], agg[:], RELU,
                                                 scale=nds_t[:, t:t + 1])
                        tp = ps_tr.tile([P, P], gdt, space="PSUM", tag="tr")
                        nc.tensor.transpose(tp[:], t2[:], ident[:])
                        t2T = ep.tile([P, P], gdt, tag="t2T")
                        nc.scalar.activation(t2T[:], tp[:], COPY)
                        dense_mm(t2T[:], w2_t, ag2_in, t)
                    else:
                        y = ep.tile([P, D], f32, tag="y")
                        if with_bias:
                            z = ep.tile([P, D], f32, tag="z")
                            nc.vector.tensor_tensor(
                                out=z[:], in0=agg[:],
                                in1=nd_t[:, t:t + 1].to_broadcast([P, D]),
                                op=mybir.AluOpType.mult)
                            nc.vector.tensor_add(out=z[:], in0=z[:], in1=b2_t[:])
                            nc.scalar.activation(y[:], z[:], RELU)
                        else:
                            nc.scalar.activation(y[:], agg[:], RELU,
                                                 scale=nd_t[:, t:t + 1])
                        rows = min(SHARD - t * P, P)
                        nc.sync.dma_start(out=y_out[t * P:t * P + rows, :],
                                          in_=y[:rows, :])

            phases = int(os.environ.get("CCAS_PHASES", "5"))
            if phases >= 3:
                agg_phase(hw1_frag, layer=1)
            if phases >= 4:
                for k in range(NW):
                    nc.gpsimd.collective_compute(
                        "AllGather", mybir.AluOpType.bypass,
                        replica_groups=[list(range(NC))],
                        ins=[ag2_in[k * FR:(k + 1) * FR, :]], outs=[hw2_frag[k][:]])
            if phases >= 5:
                agg_phase(hw2_frag, layer=2)

    nc.compile()
    return nc


def kernel(x, W1, b1, W2, b2, src, dst):
    from concourse.bass_utils import run_bass_kernel_spmd

    src = np.asarray(src).astype(np.int64)
    dst = np.asarray(dst).astype(np.int64)
    x = np.asarray(x, dtype=np.float32)
    W1 = np.asarray(W1, dtype=np.float32)
    W2 = np.asarray(W2, dtype=np.float32)
    b1 = np.asarray(b1, dtype=np.float32)
    b2 = np.asarray(b2, dtype=np.float32)

    plan, data = _plan(src, dst)
    with_bias = bool(np.any(b1) or np.any(b2))
    use_bf16 = os.environ.get("CCAS_DT", "f32") == "bf16"

    key = (with_bias, use_bf16, os.environ.get("CCAS_PHASES", "5"),
           repr(plan["calls"]), repr(plan["ktile"]))
    key = hash(key)
    if key not in _cache:
        _cache[key] = _build(plan, with_bias, use_bf16)
    nc = _cache[key]

    wdt = BF16 if use_bf16 else np.float32
    in_maps = []
    for c in range(NC):
        xp = np.zeros((SHARD_PAD, D), np.float32)
        xp[:SHARD] = x[c * SHARD:(c + 1) * SHARD]
        m = dict(
            x_in=xp,
            w1_in=W1.astype(wdt),
            w2_in=W2.astype(wdt),
            idx_in=np.tile(data["idx_all"][c], (8, 1)),
            cnt_in=data["cnt_all"][c],
            dstl_in=data["dstl_all"][c],
            ns_in=data["ns_tm"][c],
            nd_in=data["nd_tm"][c],
            nds_in=data["nd_tm"][c] * data["ns_tm"][c],
        )
        if with_bias:
            m["b1_in"] = np.broadcast_to(b1, (P, D)).astype(np.float32).copy()
            m["b2_in"] = np.broadcast_to(b2, (P, D)).astype(np.float32).copy()
        in_maps.append(m)

    prof_dir = os.environ.get("CCAS_PROFILE_DIR")
    if prof_dir:
        import sys, types
        if "antenv.axon_hooks" not in sys.modules:
            import antenv
            mod = types.ModuleType("antenv.axon_hooks")
            mod._hook = None
            mod.set_axon_ntff_profile_hook = lambda h: setattr(mod, "_hook", h)
            mod.get_axon_ntff_profile_hook = lambda: mod._hook
            sys.modules["antenv.axon_hooks"] = mod
            antenv.axon_hooks = mod
            from trn_agent_boot.trn_boot import _ntff_profile_via_ctypes
            mod.set_axon_ntff_profile_hook(
                _ntff_profile_via_ctypes("/opt/axon/libaxon_pjrt.so"))
        from antenv.axon_hooks import get_axon_ntff_profile_hook
        res = run_bass_kernel_spmd(nc, in_maps, core_ids=list(range(NC)))
        hook = get_axon_ntff_profile_hook()
        with hook(prof_dir, list(range(NC))):
            res = run_bass_kernel_spmd(nc, in_maps, core_ids=list(range(NC)))
    else:
        res = run_bass_kernel_spmd(nc, in_maps, core_ids=list(range(NC)))

    return np.concatenate([res.results[c]["y_out"] for c in range(NC)], axis=0)

